# revision 1
# baseline (speedup 1.0000x reference)
"""ConvVMamba TRN2 Bass kernel.

Sharding: data-parallel over batch. B=8 -> one image per NeuronCore, all
weights replicated, no collectives.

Per-core layout: channels on SBUF partitions (C=96), pixels on the free dim
(L=64*64=4096).
  - 1x1 convs / projections: PE matmuls (lhsT = W^T, rhs = activations).
  - depthwise 7x7 / 3x3: PE accumulating matmuls with per-tap diagonal lhsT
    over a zero-padded image buffer.
  - LayerNorm over channels: partition reduction via ones-vector matmul into
    [8,512] stats, tiny stats math, K=1 ones-matmul broadcast back.
  - selective scan (d_state=1): DVE tensor_tensor_scan (state = a*state + b),
    reversed directions via negative-step APs, transposed directions by
    keeping the whole per-direction pipeline in w-major order.
Branch tensors in bf16, residual stream in fp32.
"""

import sys
import numpy as np

sys.path.insert(0, "/opt/trn_rl_repo")

import ml_dtypes  # noqa: E402
import concourse.bass as bass  # noqa: E402
import concourse.bacc as bacc  # noqa: E402
import concourse.mybir as mybir  # noqa: E402
from concourse import tile  # noqa: E402
from concourse.tile import add_dep_helper  # noqa: E402
from concourse.bass_utils import run_bass_kernel_spmd  # noqa: E402

F32 = mybir.dt.float32
F32R = mybir.dt.float32r
BF16 = mybir.dt.bfloat16
AF = mybir.ActivationFunctionType
OP = mybir.AluOpType
bfnp = ml_dtypes.bfloat16

B, C, H, W = 8, 96, 64, 64
L = H * W
R, N, K = 6, 1, 4
EPS = 1e-5
P7, P3 = 70, 66  # padded widths for 7x7 and 3x3 convs
NCHUNK = 8  # 4096 / 512
CH = 512

_CACHE = {}


def _taps(k):
    r = (k - 1) // 2
    return [(dh, dw) for dh in range(k) for dw in range(k)], r


def build_host_tensors(kw):
    """Precompute all weight/constant DRAM tensors (shared across cores)."""
    f = lambda a: np.asarray(a, np.float32)
    out = {}

    # --- fold LN gamma/beta into following 1x1 convs ---
    def fold(wname, bname, g, b):
        w = f(kw[wname])
        bb = f(kw[bname])
        return w * f(g)[None, :], bb + w @ f(b)

    fc1w, fc1b = fold("cn_fc1_w", "cn_fc1_b", kw["cn_ln_w"], kw["cn_ln_b"])
    ipw, ipb = fold("ip_w", "ip_b", kw["v_ln1_w"], kw["v_ln1_b"])
    opw, opb = fold("op_w", "op_b", kw["o_ln_w"], kw["o_ln_b"])
    mfc1w, mfc1b = fold("m_fc1_w", "m_fc1_b", kw["v_ln2_w"], kw["v_ln2_b"])
    fc2w, fc2b = f(kw["cn_fc2_w"]), f(kw["cn_fc2_b"])
    mfc2w, mfc2b = f(kw["m_fc2_w"]), f(kw["m_fc2_b"])

    # --- depthwise conv diagonals: [96, 58*96] (49 taps of 7x7, 9 of 3x3) ---
    w7 = f(kw["cn_dw_w"]).reshape(C, 49)
    w3 = f(kw["dw_w"]).reshape(C, 9)
    diag = np.zeros((C, 58 * C), np.float32)
    ar = np.arange(C)
    for t in range(49):
        diag[ar, t * C + ar] = w7[:, t]
    for t in range(9):
        diag[ar, (49 + t) * C + ar] = w3[:, t]
    out["wdiag"] = diag.astype(bfnp)

    # --- GEMM weights (lhsT layouts), bf16 ---
    out["wfc1"] = fc1w.T.astype(bfnp)  # [96, 384]
    wfc2 = np.zeros((128, 3 * C), np.float32)  # [128, 288] K-chunks
    for j in range(3):
        wfc2[:, j * C:(j + 1) * C] = fc2w[:, j * 128:(j + 1) * 128].T
    out["wfc2"] = wfc2.astype(bfnp)
    out["wip"] = ipw.T.astype(bfnp)  # [96, 96]
    out["wop"] = opw.T.astype(bfnp)
    out["wmfc1"] = mfc1w.T.astype(bfnp)
    wm2 = np.zeros((128, 3 * C), np.float32)
    for j in range(3):
        wm2[:, j * C:(j + 1) * C] = mfc2w[:, j * 128:(j + 1) * 128].T
    out["wmfc2"] = wm2.astype(bfnp)

    # x_proj lhsT [96, 32]: cols 0:8 k0, 8:16 k2, 16:24 k1, 24:32 k3
    xp = f(kw["x_proj_w"])  # [4, 8, 96]
    wxp = np.concatenate([xp[0].T, xp[2].T, xp[1].T, xp[3].T], axis=1)
    # v4 is materialized as 2*silu (tanh form); fold the 0.5 here
    out["wxp"] = (0.5 * wxp).astype(bfnp)  # [96, 32]

    # xdbl row bases within the [48, L] tile: k0@0, k2@8, k1@32, k3@40.
    # dt lhsT [48, 192]: K=16 matmuls over xdbl[0:16] / xdbl[32:48] with
    # zero rows outside each direction's 6 dt rows.
    dtw = f(kw["dt_w"])  # [4, 96, 6]
    wdt = np.zeros((48, 2 * C), np.float32)
    wdt[0:6, 0:C] = dtw[0].T
    wdt[32:38, 0:C] = dtw[1].T
    wdt[8:14, C:2 * C] = dtw[2].T
    wdt[40:46, C:2 * C] = dtw[3].T
    out["wdt"] = wdt.astype(bfnp)
    # B/C broadcast selector lhsT [48, 384]: block b*96 picks one xdbl row
    # (all-ones row -> out[m,n] = xdbl[row,n] broadcast over 96 partitions)
    sel = np.zeros((48, 4 * C), np.float32)
    sel[6, 0:C] = 0.5       # B k0 (0.5: v4 carries 2*silu)
    sel[38, 0:C] = 0.5      # B k1 (slice [32:48])
    sel[7, C:2 * C] = 1.0   # C k0
    sel[39, C:2 * C] = 1.0  # C k1
    sel[14, 2 * C:3 * C] = 0.5  # B k2
    sel[46, 2 * C:3 * C] = 0.5  # B k3
    sel[15, 3 * C:4 * C] = 1.0  # C k2
    sel[47, 3 * C:4 * C] = 1.0  # C k3
    out["sel"] = sel.astype(bfnp)

    # ones for LN partition reduce+broadcast
    out["ones96"] = np.ones((C, C), np.float32)
    out["ones96_bf"] = np.ones((C, C), bfnp)

    # per-partition scalar bank [128, NV] fp32
    A = (-np.exp(f(kw["A_logs"]))).reshape(K, C)
    Ds = f(kw["Ds"]).reshape(K, C)
    dtb = f(kw["dt_b"])  # [4, 96]
    cols = []

    def col(v, n=C):
        a = np.zeros(128, np.float32)
        a[: len(v)] = v
        cols.append(a)
        return len(cols) - 1

    ix = {}
    ix["cn_dw_b"] = col(f(kw["cn_dw_b"]))
    for j in range(3):
        ix[f"fc1b{j}"] = col(fc1b[j * 128:(j + 1) * 128])
    ix["fc2b"] = col(fc2b)
    ix["ipb"] = col(ipb)
    ix["dwb"] = col(f(kw["dw_b"]))
    ix["dwb_h"] = col(0.5 * f(kw["dw_b"]))
    for k in range(4):
        ix[f"dtb{k}"] = col(dtb[k])
        ix[f"A{k}"] = col(A[k])
    ix["Dsum"] = col(0.5 * Ds.sum(0))
    ix["eps"] = col(np.full(128, EPS, np.float32), 128)
    ix["one"] = col(np.full(128, 1.0, np.float32), 128)
    ix["opb"] = col(opb)
    for j in range(3):
        ix[f"mfc1b{j}"] = col(mfc1b[j * 128:(j + 1) * 128])
    ix["mfc2b"] = col(mfc2b)
    out["vecs"] = np.stack(cols, axis=1)  # [128, NV]
    return out, ix


def pad_image(x):
    """[96,64,64] fp32 -> padded [96,70*70] bf16."""
    xp = np.zeros((C, P7, P7), np.float32)
    xp[:, 3:3 + H, 3:3 + W] = x
    return xp.reshape(C, P7 * P7).astype(bfnp)


def r32(ap):
    return ap.bitcast(F32R)


def build_program(ix):
    nc = bacc.Bacc("TRN2", target_bir_lowering=False, debug=False)

    din = {}
    for name, shape, dt in [
        ("xpad", [C, P7 * P7], BF16),
        ("xres", [C, L], F32),
        ("wdiag", [C, 58 * C], BF16),
        ("wfc1", [C, 384], BF16),
        ("wfc2", [128, 3 * C], BF16),
        ("wip", [C, C], BF16),
        ("wop", [C, C], BF16),
        ("wmfc1", [C, 384], BF16),
        ("wmfc2", [128, 3 * C], BF16),
        ("wxp", [C, 32], BF16),
        ("wdt", [48, 2 * C], BF16),
        ("sel", [48, 4 * C], BF16),
        ("ones96", [C, C], F32R),
        ("ones96_bf", [C, C], BF16),
        ("vecs", [128, len(ix)], F32),
    ]:
        din[name] = nc.dram_tensor(name, shape, dt, kind="ExternalInput").ap()
    dout = nc.dram_tensor("out", [C, L], F32, kind="ExternalOutput").ap()

    class ActPhase:
        # Fences set-specific ACT ops so the scheduler cannot interleave
        # exp/ln-table ops with gelu-table ops (each flip costs an ACT
        # function-table reload).
        def __init__(self):
            self.prev_last = None
            self.cur_last = None
            self.cur_set = None

        def tag(self, bi, fset):
            # total order over set-specific ACT ops = emission order, so the
            # table-set phases stay contiguous in the final schedule
            inst = bi.ins
            if self.cur_last is not None:
                add_dep_helper(inst, self.cur_last, sync=True,
                               reason="act table-set phase fence")
            self.cur_last = inst
            return bi

    ph = ActPhase()

    with tile.TileContext(nc) as tc:
        from contextlib import ExitStack

        with ExitStack() as ctx:
            const = ctx.enter_context(tc.tile_pool(name="const", bufs=1))
            bigp = ctx.enter_context(tc.tile_pool(name="big", bufs=1))
            scanp = ctx.enter_context(tc.tile_pool(name="scan", bufs=4))
            hcp = ctx.enter_context(tc.tile_pool(name="hc", bufs=2))
            accp = ctx.enter_context(tc.tile_pool(name="acc", bufs=2))
            chk = ctx.enter_context(tc.tile_pool(name="chk", bufs=3))
            ps = ctx.enter_context(tc.tile_pool(name="ps", bufs=2, space="PSUM"))
            psf1 = ctx.enter_context(tc.tile_pool(name="psf1", bufs=1, space="PSUM"))

            # ---- load constants ----
            cc = {}
            for name, ap in din.items():
                if name in ("xpad", "xres"):
                    continue
                t = const.tile(list(ap.shape), ap.dtype, tag=name)
                nc.sync.dma_start(t[:], ap)
                cc[name] = t
            # Route the bias bank through an ACT copy: the ACT instruction
            # encoding has a single sync-wait slot, so later ACT ops must not
            # need a DMA wait on top of their PSUM wait.
            nv = len(ix)
            vecs_sb = const.tile([128, nv], F32, tag="vecs_sb")
            nc.scalar.activation(vecs_sb[:], cc["vecs"][:], AF.Copy)
            # dummy reader absorbs the same-engine RAW wait on vecs_sb so
            # later ACT ops keep a single wait slot for their PSUM input
            scr = const.tile([128, 1], F32, tag="scr")
            nc.scalar.activation(scr[:], vecs_sb[:, 0:1], AF.Copy)
            V = lambda key: vecs_sb[:, ix[key]:ix[key] + 1]
            V96 = lambda key: vecs_sb[:C, ix[key]:ix[key] + 1]

            xpad = bigp.tile([C, P7 * P7], BF16, tag="pad")
            nc.sync.dma_start(xpad[:], din["xpad"])
            xres = bigp.tile([C, L], F32, tag="xres")
            nc.sync.dma_start(xres[:], din["xres"])

            # =============== helpers ===============
            def dwconv_psum(src_pad, Wp, ktaps, diag_off, jchunk, ptag):
                """depthwise conv for output rows [8*j, 8*j+8) -> psum [96,512]"""
                taps, rr = _taps(ktaps)
                pt = ps.tile([C, CH], F32, tag=ptag)
                src3 = src_pad[:].rearrange("c (h w) -> c h w", w=Wp)
                r0 = jchunk * 8
                nt = len(taps)
                for t, (dh, dw) in enumerate(taps):
                    rhs = src3[:, r0 + dh:r0 + dh + 8, dw:dw + W]
                    nc.tensor.matmul(
                        pt[:],
                        cc["wdiag"][:, (diag_off + t) * C:(diag_off + t + 1) * C],
                        rhs,
                        start=(t == 0),
                        stop=(t == nt - 1),
                    )
                return pt

            def ln_norm_chunk(src_chunk, src_f32, out_chunk):
                """out = (x - mean_c) / sqrt(var_c + eps) for one 512-chunk.

                Fused partition reduce+broadcast via all-ones [96,96] lhsT.
                """
                mb = ps.tile([C, CH], F32, tag="gen")
                if src_f32:
                    nc.tensor.matmul(mb[:], cc["ones96"][:], r32(src_chunk),
                                     start=True, stop=True)
                else:
                    nc.tensor.matmul(mb[:], cc["ones96_bf"][:], src_chunk,
                                     start=True, stop=True)
                d = chk.tile([C, CH], F32, tag="lnd")
                nc.vector.scalar_tensor_tensor(d[:], mb[:], -1.0 / C, src_chunk,
                                               OP.mult, OP.add)
                dsq = chk.tile([C, CH], BF16, tag="sq")
                nc.scalar.activation(dsq[:], d[:], AF.Square)
                vb = ps.tile([C, CH], F32, tag="gen2")
                nc.tensor.matmul(vb[:], cc["ones96_bf"][:], dsq[:],
                                 start=True, stop=True)
                # rstd = exp(-0.5*ln(v/C + eps)) -- keeps LN inside the
                # exp/ln ACT table set (no sqrt table, no DVE reciprocal)
                lnv = chk.tile([C, CH], F32, tag="sd")
                ph.tag(nc.scalar.activation(lnv[:], vb[:], AF.Ln,
                                            scale=1.0 / C, bias=V96("eps")),
                       "expln")
                rstd = chk.tile([C, CH], F32, tag="rstd")
                ph.tag(nc.scalar.activation(rstd[:], lnv[:], AF.Exp,
                                            scale=-0.5), "expln")
                nc.vector.tensor_tensor(out_chunk, d[:], rstd[:], OP.mult)

            def mlp_block(src_tile, src_f32, wf1, wf2, b1pfx, b2key, res_tile,
                          out_tile, round_out=False):
                """out = res + fc2(gelu(fc1(LN(src)))) ; all chunked.

                LN chunks are materialized fully first so the exp/ln ACT
                table phase doesn't interleave with the gelu table phase.
                """
                xnf = scanp.tile([C, L], BF16, tag="sc", name="xnf")
                for j in range(NCHUNK):
                    ln_norm_chunk(src_tile[:, j * CH:(j + 1) * CH], src_f32,
                                  xnf[:, j * CH:(j + 1) * CH])
                for j in range(NCHUNK):
                    xn = xnf[:, j * CH:(j + 1) * CH]
                    gs = []
                    for mm in range(3):
                        f1 = psf1.tile([128, CH], F32, tag=f"f1_{mm}")
                        nc.tensor.matmul(f1[:], cc[wf1][:, mm * 128:(mm + 1) * 128],
                                         xn, start=True, stop=True)
                        g = chk.tile([128, CH], BF16, tag=f"g{mm}")
                        ph.tag(nc.scalar.activation(g[:], f1[:], AF.Gelu,
                                                    bias=V(f"{b1pfx}{mm}")),
                               "gelu")
                        gs.append(g)
                    f2 = ps.tile([C, CH], F32, tag="gen")
                    for mm in range(3):
                        nc.tensor.matmul(f2[:], cc[wf2][:, mm * C:(mm + 1) * C],
                                         gs[mm][:], start=(mm == 0), stop=(mm == 2))
                    oap = out_tile[:, j * CH:(j + 1) * CH]
                    if round_out:
                        oap = oap.bitcast(F32R)
                    nc.vector.scalar_tensor_tensor(
                        oap, f2[:], V96(b2key),
                        res_tile[:, j * CH:(j + 1) * CH], OP.add, OP.add)

            # =============== ConvNeXt block ===============
            hsb = bigp.tile([C, L], BF16, tag="bufA")
            for j in range(NCHUNK):
                pc = dwconv_psum(xpad, P7, 7, 0, j, "gen")
                nc.scalar.activation(hsb[:, j * CH:(j + 1) * CH], pc[:],
                                     AF.Identity, bias=V96("cn_dw_b"))
            x1 = bigp.tile([C, L], F32, tag="x1")
            mlp_block(hsb, False, "wfc1", "wfc2", "fc1b", "fc2b", xres, x1,
                      round_out=True)

            # =============== SS2D: LN1 + in_proj + dwconv3 + silu ==========
            v2pad = bigp.tile([C, P3 * P3], BF16, tag="pad2")
            nc.gpsimd.memset(v2pad[:], 0.0)
            v2int = v2pad[:].rearrange("c (h w) -> c h w", w=P3)
            xn1f = scanp.tile([C, L], BF16, tag="sc", name="xn1f")
            for j in range(NCHUNK):
                ln_norm_chunk(x1[:, j * CH:(j + 1) * CH], True,
                              xn1f[:, j * CH:(j + 1) * CH])
            for j in range(NCHUNK):
                pv = ps.tile([C, CH], F32, tag="gen")
                nc.tensor.matmul(pv[:], cc["wip"][:],
                                 xn1f[:, j * CH:(j + 1) * CH], start=True,
                                 stop=True)
                dst = v2int[:, 1 + j * 8:1 + (j + 1) * 8, 1:1 + W]
                nc.scalar.activation(dst, pv[:], AF.Identity, bias=V96("ipb"))
            v4 = bigp.tile([C, L], BF16, tag="bufA")
            for j in range(NCHUNK):
                pc = dwconv_psum(v2pad, P3, 3, 49, j, "gen")
                th = chk.tile([C, CH], BF16, tag="sq")
                ph.tag(nc.scalar.activation(th[:], pc[:], AF.Tanh, scale=0.5,
                                            bias=V96("dwb_h")), "gelu")
                xb = chk.tile([C, CH], BF16, tag="lnd")
                nc.scalar.activation(xb[:], pc[:], AF.Identity, bias=V96("dwb"))
                nc.vector.scalar_tensor_tensor(v4[:, j * CH:(j + 1) * CH],
                                               th[:], 1.0, xb[:], OP.add,
                                               OP.mult)

            # =============== cross-scan projections ===============
            # xdbl [48, 4096]: rows 0:8 k0, 8:16 k2 (l-major); 32:40 k1,
            # 40:48 k3 (w-major).  PSUM/ACT lanes are partition-aligned, so
            # the k13 matmul writes at PSUM base partition 32 directly.
            xdbl = bigp.tile([48, L], BF16, tag="xdbl")
            v4T = v4[:].rearrange("c (h w) -> c h w", w=W).transpose([0, 2, 1])
            for j in range(NCHUNK):
                p1 = ps.tile([48, CH], F32, tag="gen")
                nc.tensor.matmul(p1[0:16, :], cc["wxp"][:, 0:16],
                                 v4[:, j * CH:(j + 1) * CH], start=True, stop=True)
                rhsT = v4T[:, j * 8:(j + 1) * 8, :]
                nc.tensor.matmul(p1[32:48, :], cc["wxp"][:, 16:32], rhsT,
                                 start=True, stop=True)
                nc.scalar.activation(xdbl[:, j * CH:(j + 1) * CH],
                                     p1[:], AF.Copy)

            # =============== per-direction scan ===============
            accs = {}
            for k in [0, 2, 1, 3]:
                # xdbl 16-row block for this direction pair and lhsT columns
                blk0 = 0 if k in (0, 2) else 32
                xblk = lambda j: xdbl[blk0:blk0 + 16, j * CH:(j + 1) * CH]
                wblk = lambda t, c0: t[blk0:blk0 + 16, c0 * C:(c0 + 1) * C]
                dtcol = 0 if k in (0, 1) else 1
                bcol = 0 if k in (0, 1) else 2  # sel B col block
                ccol = 1 if k in (0, 1) else 3  # sel C col block
                uview = v4[:] if k in (0, 2) else v4T
                delta = scanp.tile([C, L], BF16, tag="sc")
                for j in range(NCHUNK):
                    pd = ps.tile([C, CH], F32, tag="gen")
                    nc.tensor.matmul(pd[:], wblk(cc["wdt"][:], dtcol), xblk(j),
                                     start=True, stop=True)
                    # softplus(z) = ln(1 + exp(z)) (no native softplus table)
                    ez = chk.tile([C, CH], F32, tag="sd")
                    ph.tag(nc.scalar.activation(ez[:], pd[:], AF.Exp,
                                                bias=V96(f"dtb{k}")), "expln")
                    ph.tag(nc.scalar.activation(delta[:, j * CH:(j + 1) * CH],
                                                ez[:], AF.Ln, bias=V96("one")),
                           "expln")
                dA = scanp.tile([C, L], BF16, tag="sc")
                ph.tag(nc.scalar.activation(dA[:], delta[:], AF.Exp,
                                            scale=V96(f"A{k}")), "expln")
                bso = scanp.tile([C, L], BF16, tag="sc")
                for j in range(NCHUNK):
                    bb = ps.tile([C, CH], F32, tag="gen2")
                    nc.tensor.matmul(bb[:], wblk(cc["sel"][:], bcol), xblk(j),
                                     start=True, stop=True)
                    du = chk.tile([C, CH], BF16, tag="du")
                    nc.vector.tensor_tensor(du[:],
                                            delta[:, j * CH:(j + 1) * CH],
                                            uview[:, j * 8:(j + 1) * 8, :]
                                            if k in (1, 3)
                                            else uview[:, j * CH:(j + 1) * CH],
                                            OP.mult)
                    nc.vector.tensor_tensor(bso[:, j * CH:(j + 1) * CH], du[:],
                                            bb[:], OP.mult)
                h = scanp.tile([C, L], BF16, tag="sc")
                if k in (0, 1):
                    nc.vector.tensor_tensor_scan(h[:], dA[:], bso[:], 0.0,
                                                 OP.mult, OP.add)
                else:
                    nc.vector.tensor_tensor_scan(h[:][:, ::-1], dA[:][:, ::-1],
                                                 bso[:][:, ::-1], 0.0,
                                                 OP.mult, OP.add)
                # y_k = h * Cs_b  (+ accumulate into l-major / w-major accs)
                if k in (0, 1):
                    dst = hcp.tile([C, L], BF16, tag="hc", name=f"hc{k}")
                else:
                    dst = accp.tile([C, L], BF16, tag="acc", name=f"acc{k}")
                for j in range(NCHUNK):
                    cb = ps.tile([C, CH], F32, tag="gen2")
                    nc.tensor.matmul(cb[:], wblk(cc["sel"][:], ccol), xblk(j),
                                     start=True, stop=True)
                    if k in (0, 1):
                        nc.vector.tensor_tensor(dst[:, j * CH:(j + 1) * CH],
                                                h[:, j * CH:(j + 1) * CH],
                                                cb[:], OP.mult)
                    else:
                        tmp = chk.tile([C, CH], BF16, tag="du")
                        nc.vector.tensor_tensor(tmp[:],
                                                h[:, j * CH:(j + 1) * CH],
                                                cb[:], OP.mult)
                        nc.vector.tensor_tensor(dst[:, j * CH:(j + 1) * CH],
                                                accs[k - 2][:,
                                                            j * CH:(j + 1) * CH],
                                                tmp[:], OP.add)
                accs[k] = dst

            # =============== cross-merge + D*u + LN + out_proj =============
            preln = hcp.tile([C, L], BF16, tag="hc")
            accT = accs[3][:].rearrange("c (w h) -> c w h", w=W).transpose(
                [0, 2, 1])
            for j in range(NCHUNK):
                t2 = chk.tile([C, CH], BF16, tag="du")
                nc.vector.tensor_tensor(t2[:],
                                        accs[2][:, j * CH:(j + 1) * CH],
                                        accT[:, j * 8:(j + 1) * 8, :], OP.add)
                nc.vector.scalar_tensor_tensor(
                    preln[:, j * CH:(j + 1) * CH],
                    v4[:, j * CH:(j + 1) * CH], V96("Dsum"), t2[:],
                    OP.mult, OP.add)
            x2 = bigp.tile([C, L], F32, tag="x2")
            ynf = scanp.tile([C, L], BF16, tag="sc", name="ynf")
            for j in range(NCHUNK):
                ln_norm_chunk(preln[:, j * CH:(j + 1) * CH], False,
                              ynf[:, j * CH:(j + 1) * CH])
            for j in range(NCHUNK):
                po = ps.tile([C, CH], F32, tag="gen")
                nc.tensor.matmul(po[:], cc["wop"][:],
                                 ynf[:, j * CH:(j + 1) * CH], start=True,
                                 stop=True)
                nc.vector.scalar_tensor_tensor(x2[:, j * CH:(j + 1) * CH]
                                               .bitcast(F32R), po[:],
                                               V96("opb"),
                                               x1[:, j * CH:(j + 1) * CH],
                                               OP.add, OP.add)

            # =============== MLP block ===============
            outsb = bigp.tile([C, L], F32, tag="x1")
            mlp_block(x2, True, "wmfc1", "wmfc2", "mfc1b", "mfc2b", x2, outsb)
            nc.sync.dma_start(dout, outsb[:])

    nc.compile()
    return nc


def get_program_and_inputs(inputs):
    key = "prog"
    host, ix = build_host_tensors(inputs)
    if key not in _CACHE:
        _CACHE[key] = build_program(ix)
    nc = _CACHE[key]
    x = np.asarray(inputs["x"], np.float32)
    in_maps = []
    for b in range(B):
        m = {k: v for k, v in host.items()}
        m["xpad"] = pad_image(x[b])
        m["xres"] = x[b].reshape(C, L).astype(np.float32)
        in_maps.append(m)
    return nc, in_maps


def kernel(**inputs):
    nc, in_maps = get_program_and_inputs(inputs)
    res = run_bass_kernel_spmd(nc, in_maps, list(range(B)))
    out = np.stack([res.results[b]["out"].reshape(C, H, W) for b in range(B)])
    return out.astype(np.float32)


if __name__ == "__main__":
    # smoke build
    host, ix = build_host_tensors(
        {k: np.zeros(s, np.float32) for k, s in [  # noqa

            ("x", (B, C, H, W)), ("cn_dw_w", (C, 7, 7)), ("cn_dw_b", (C,)),
            ("cn_ln_w", (C,)), ("cn_ln_b", (C,)), ("cn_fc1_w", (4 * C, C)),
            ("cn_fc1_b", (4 * C,)), ("cn_fc2_w", (C, 4 * C)), ("cn_fc2_b", (C,)),
            ("v_ln1_w", (C,)), ("v_ln1_b", (C,)), ("ip_w", (C, C)),
            ("ip_b", (C,)), ("dw_w", (C, 3, 3)), ("dw_b", (C,)),
            ("x_proj_w", (K, R + 2 * N, C)), ("dt_w", (K, C, R)),
            ("dt_b", (K, C)), ("A_logs", (K * C, N)), ("Ds", (K * C,)),
            ("o_ln_w", (C,)), ("o_ln_b", (C,)), ("op_w", (C, C)),
            ("op_b", (C,)), ("v_ln2_w", (C,)), ("v_ln2_b", (C,)),
            ("m_fc1_w", (4 * C, C)), ("m_fc1_b", (4 * C,)),
            ("m_fc2_w", (C, 4 * C)), ("m_fc2_b", (C,)),
        ]})
    nc = build_program(ix)
    print("program built OK:", len(list(nc.all_instructions())), "instructions")

